# revision 20
# baseline (speedup 1.0000x reference)
"""Dot-product attention (no softmax) on 8 TRN2 NeuronCores.

out[b,h] = (q[b,h] @ k[b,h].T) @ v[b,h]  for q,k,v [B,H,L,D] = [2,16,2048,64] f32.

Strategy: matmul associativity -> out = q @ (k.T @ v). KV = k.T@v is [64,64]
per head, so the problem collapses from O(L^2 D) to O(L D^2) flops and becomes
purely memory bound (6 MiB in / 2 MiB out per core; ~20us at the measured
~420 GB/s per-core DMA rate).

Sharding: the 32 (b,h) attention instances are independent; each of the 8
cores handles 4 consecutive heads of the flattened (b*h) axis. No collectives.

v7 (trace-driven; baseline 41.5us -> v4 38.6us -> this). Measured constraints
this schedule is built around:
- A DMA's completion semaphore fires ~1.5-2.5us after its last byte (HBM
  receipt under load), so every dependency boundary on a load costs that
  latency on top of stream position.
- fp32 matmuls cost 4 cycles/row vs 1 for bf16 (fp32 transposes 2cyc via a
  LOW/HIGH double pass), so k,v are cast f32->bf16 on the DVE (2 elem/cycle/
  lane from SBUF); q is consumed f32 by the PE transpose whose PSUM->SBUF
  copy writes bf16 (no separate q cast).
- The HAM clock gate halves the PE clock after any ~3.4us idle window; the
  warm-up is sized to bridge to the first load's semaphore (~24 matmuls) and
  one mid bundle covers the one unavoidable data gap.

Schedule: loads [qkv0, qkv1, kv2, kv3, q2, q3a, q3b]. Heads 0/1 run full
chains as their fused loads land; heads 2/3's cast->KV->KV2 chains complete
under the load stream (kv planes arrive mid-stream), so after the last q
bytes only a short transpose -> out-matmul -> copy -> store chain remains,
split into half-heads with the PSUM->SBUF copies alternating between the
scalar and vector engines so consecutive chunks overlap. PSUM->SBUF copies
are batched 4-wide (one 2 KiB bank, amortizes ~150ns/instruction).

Per-core layout trick: a head's [2048, 64] tensor is viewed as [128, 16, 64]
(partition p holds rows 16p..16p+15, 4 KiB contiguous DRAM per partition, so
every DMA is fully coalesced). The KV reduction over L is order-independent,
and the same interleaved row mapping flows through transpose -> matmul ->
store unchanged.
"""

import sys

if "/opt/trn_rl_repo" not in sys.path:
    sys.path.insert(0, "/opt/trn_rl_repo")

from contextlib import ExitStack

import numpy as np

import concourse.bass as bass
import concourse.tile as tile
from concourse import bacc, mybir
from concourse.bass_utils import run_bass_kernel_spmd

B, H, L, D = 2, 16, 2048, 64
N_CORES = 8
HPC = (B * H) // N_CORES  # heads per core = 4
P = 128
J = L // P  # 16 row-slots per partition
F32 = mybir.dt.float32
BF16 = mybir.dt.bfloat16


def _body(ctx: ExitStack, tc: tile.TileContext, o_d, qkv_d):
    nc = tc.nc

    const_pool = ctx.enter_context(tc.tile_pool(name="const", bufs=1))
    in_pool = ctx.enter_context(tc.tile_pool(name="in", bufs=4))
    kvb_pool = ctx.enter_context(tc.tile_pool(name="kvb", bufs=4))
    qt_pool = ctx.enter_context(tc.tile_pool(name="qt", bufs=8))
    kv_pool = ctx.enter_context(tc.tile_pool(name="kv", bufs=4))
    out_pool = ctx.enter_context(tc.tile_pool(name="out", bufs=4))
    psum_kv = ctx.enter_context(tc.tile_pool(name="psum_kv", bufs=1, space="PSUM"))
    psum_s = ctx.enter_context(tc.tile_pool(name="psum_s", bufs=1, space="PSUM"))
    psum_t = ctx.enter_context(tc.tile_pool(name="psum_t", bufs=2, space="PSUM"))
    psum_o = ctx.enter_context(tc.tile_pool(name="psum_o", bufs=2, space="PSUM"))
    psum_w = ctx.enter_context(tc.tile_pool(name="psum_w", bufs=1, space="PSUM"))

    qkv_sbs = [
        in_pool.tile([P, 3, J, D], F32, tag="qkv", name=f"qkv{h}") for h in range(HPC)
    ]
    kv_sbs = [
        kvb_pool.tile([P, 2, J, D], BF16, tag="kvb", name=f"kvb{h}")
        for h in range(HPC)
    ]

    def qkv_view(h):
        # [3, L, D] f32 in DRAM -> [p, t, j, d]; per partition chunks of
        # 4 KiB (q/k/v planes), fully coalesced descriptors.
        return qkv_d[h].rearrange("t (p j) d -> p t j d", p=P)

    # All loads issued up front on the sync queue (HWDGE); stores are emitted
    # later so their semaphore waits cannot delay a load. kv planes of heads
    # 2/3 arrive mid-stream (their chains finish under the stream); only the
    # q planes land late, and their post-load chain is short.
    # q planes of heads 2/3 load BEFORE their kv planes: the transposes then
    # run warm mid-stream (a cold fp32 transpose costs 420ns vs 213), while
    # the kv-side tail (cast -> KV -> fixup) is clock-insensitive — the KV
    # matmuls pipeline at ~53ns issue cadence even throttled and the fixup
    # runs on ACT/DVE.
    nc.sync.dma_start(qkv_sbs[0][:], qkv_view(0))
    nc.sync.dma_start(qkv_sbs[1][:], qkv_view(1))
    nc.sync.dma_start(qkv_sbs[2][:, 0], qkv_view(2)[:, 0])  # q2
    nc.sync.dma_start(qkv_sbs[3][:, 0], qkv_view(3)[:, 0])  # q3
    # kv planes in j-halves: each half's completion semaphore (~1.5us after
    # its last byte) overlaps the next half's stream, so the casts start
    # ~1.2us earlier than with monolithic kv loads.
    nc.sync.dma_start(qkv_sbs[2][:, 1:3, 0:8], qkv_view(2)[:, 1:3, 0:8])
    nc.sync.dma_start(qkv_sbs[2][:, 1:3, 8:J], qkv_view(2)[:, 1:3, 8:J])
    nc.sync.dma_start(qkv_sbs[3][:, 1:3, 0:8], qkv_view(3)[:, 1:3, 0:8])
    nc.sync.dma_start(qkv_sbs[3][:, 1:3, 8:J], qkv_view(3)[:, 1:3, 8:J])

    # HAM warm-up: dense bf16 matmuls bridge from kernel start to the first
    # load's completion semaphore (~8 cold + 16 warm ~= 7us) so the PE runs
    # at 2.4 GHz when real work starts. Results are never read.
    warm_in = const_pool.tile([P, 4 * P], BF16)
    nc.vector.memset(warm_in[:], 0.0)
    warm_ps = psum_w.tile([P, 4 * P], F32)

    def warm_bundle(n):
        for _ in range(n):
            nc.tensor.matmul(
                warm_ps[:], warm_in[:, 0:P], warm_in[:], start=True, stop=True
            )

    warm_bundle(24)

    # Identity (f32, matching q's dtype) for PE transposes.
    ident = const_pool.tile([P, P], F32)
    nc.gpsimd.memset(ident[:], 0.0)
    nc.gpsimd.affine_select(
        out=ident[:],
        in_=ident[:],
        compare_op=mybir.AluOpType.not_equal,
        fill=1.0,
        base=0,
        pattern=[[-1, P]],
        channel_multiplier=1,
    )

    # ones_dbl[p, m] = 1 iff p == m (mod 64): one matmul against it both sums
    # the two column-tiled KV halves and replicates the result to partitions
    # 64..127 (the odd-slot block of KV2).
    ones_dbl = const_pool.tile([P, P], BF16)
    nc.gpsimd.memset(ones_dbl[:], 0.0)
    for off in (-64, 0, 64):
        nc.gpsimd.affine_select(
            out=ones_dbl[:],
            in_=ones_dbl[:],
            compare_op=mybir.AluOpType.not_equal,
            fill=1.0,
            base=-off,
            pattern=[[-1, P]],
            channel_multiplier=1,
        )

    qts_all = [[None, None] for _ in range(HPC)]
    kv2s = [None] * HPC

    def emit_T_group(h, g, dve=False):
        """Transpose q_h slab-pairs 4g..4g+3 into one PSUM bank, then one
        batched copy (f32 PSUM -> bf16 SBUF) on ACT (or DVE)."""
        q_sb = qkv_sbs[h][:, 0]
        qt_ps = psum_t.tile([P, 4, P], F32, tag="qt_ps")
        for i in range(4):
            jp = 4 * g + i
            nc.tensor.matmul(
                qt_ps[:, i],
                q_sb[:, 2 * jp : 2 * jp + 2],
                ident[:],
                is_transpose=True,
                start=True,
                stop=True,
                skip_group_check=True,
            )
        qt_sb = qt_pool.tile([P, 4, P], BF16, tag="qt", name=f"qt{h}_{g}")
        if dve:
            nc.vector.tensor_copy(qt_sb[:], qt_ps[:])
        else:
            nc.scalar.activation(
                qt_sb[:], qt_ps[:], mybir.ActivationFunctionType.Identity
            )
        qts_all[h][g] = qt_sb

    def emit_cast(h):
        # k,v f32 -> bf16 on the DVE, in two halves so the first half of the
        # KV matmuls starts ~0.6us earlier; emitted separately from the KV
        # matmuls so consecutive heads' casts queue back-to-back on the DVE.
        nc.vector.tensor_copy(
            kv_sbs[h][:, :, 0 : J // 2], qkv_sbs[h][:, 1:3, 0 : J // 2]
        )
        nc.vector.tensor_copy(
            kv_sbs[h][:, :, J // 2 : J], qkv_sbs[h][:, 1:3, J // 2 : J]
        )

    def emit_kv_chain(h):
        """KV accumulation from the bf16 cast, KV2 = blockdiag(KV, KV).
        The small fixup copies run on ACT so the DVE stays free for casts."""
        k_sb = kv_sbs[h][:, 0]
        v_sb = kv_sbs[h][:, 1]

        # KV = k.T @ v, column-tiled: even j-slots accumulate into PE columns
        # 0..63, odd slots into 64..127, so pair matmuls run concurrently.
        kv_ps = psum_kv.tile([P, D], F32)
        for jp in range(J // 2):
            nc.tensor.matmul(
                kv_ps[0:D],
                k_sb[:, 2 * jp],
                v_sb[:, 2 * jp],
                start=(jp == 0),
                stop=(jp == J // 2 - 1),
                tile_position=(0, 0),
                skip_group_check=True,
            )
            nc.tensor.matmul(
                kv_ps[D : 2 * D],
                k_sb[:, 2 * jp + 1],
                v_sb[:, 2 * jp + 1],
                start=(jp == 0),
                stop=(jp == J // 2 - 1),
                tile_position=(0, D),
                skip_group_check=True,
            )
        kv_raw = kv_pool.tile([P, D], BF16, tag="kv_raw", name=f"kvr{h}")
        nc.scalar.activation(
            kv_raw[:], kv_ps[:], mybir.ActivationFunctionType.Identity
        )
        kv_st_ps = psum_s.tile([P, D], F32, tag="kv_st", name=f"kvs{h}")
        nc.tensor.matmul(kv_st_ps[:], ones_dbl[:], kv_raw[:], start=True, stop=True)
        kv2 = kv_pool.tile([P, P], BF16, tag="kv2", name=f"kv2_{h}")
        nc.gpsimd.memset(kv2[:], 0.0)
        nc.scalar.activation(
            kv2[0:D, 0:D], kv_st_ps[0:D], mybir.ActivationFunctionType.Identity
        )
        nc.vector.tensor_copy(kv2[D : 2 * D, D : 2 * D], kv_st_ps[D : 2 * D])
        kv2s[h] = kv2

    out_sbs = [
        out_pool.tile([P, J, D], F32, tag="o", name=f"o{h}") for h in range(HPC)
    ]

    def emit_O_group(h, g, dve=False):
        """Out matmuls for slab-pairs 4g..4g+3, then a batched copy."""
        out_sb = out_sbs[h]
        o_ps = psum_o.tile([P, 8, D], F32, tag="o_ps")
        for i in range(4):
            nc.tensor.matmul(
                o_ps[:, 2 * i : 2 * i + 2],
                qts_all[h][g][:, i],
                kv2s[h][:],
                start=True,
                stop=True,
                skip_group_check=True,
            )
        half = slice(8 * g, 8 * g + 8)
        if dve:
            nc.vector.tensor_copy(out_sb[:, half], o_ps[:])
        else:
            nc.scalar.activation(
                out_sb[:, half], o_ps[:], mybir.ActivationFunctionType.Identity
            )

    # Heads 0/1: full chains as their fused loads land (transposes before the
    # KV matmuls so the PE works during the DVE cast). One bundle fills the
    # data gap between O0 and head 2's chain.
    emit_T_group(0, 0)
    emit_T_group(0, 1)
    emit_cast(0)
    emit_kv_chain(0)
    emit_T_group(1, 0)
    emit_T_group(1, 1)
    emit_cast(1)
    emit_kv_chain(1)
    emit_O_group(0, 0)
    emit_O_group(0, 1)
    warm_bundle(12)
    emit_O_group(1, 0)
    emit_O_group(1, 1)
    # Heads 2/3: transposes first (q planes arrive mid-stream, PE still
    # warm), then the kv chains in load order; every engine queue sees its
    # tail work in readiness order, so nothing blocks behind a later wait.
    emit_T_group(2, 0, dve=False)
    emit_T_group(2, 1, dve=False)
    emit_T_group(3, 0, dve=False)
    emit_T_group(3, 1, dve=False)
    warm_bundle(10)
    emit_cast(2)
    emit_kv_chain(2)
    emit_cast(3)
    emit_O_group(2, 0, dve=False)
    emit_O_group(2, 1, dve=True)
    emit_kv_chain(3)
    emit_O_group(3, 0, dve=False)
    emit_O_group(3, 1, dve=True)

    # Stores, clock-gated to start only once the (pure) load stream has
    # drained: mixed load+store traffic measured ~360 B/ns vs ~420 for a
    # single direction, and interleaved stores also delay every load's
    # completion semaphore (observed +3us), which paces the whole pipeline.
    # 22us on the engine clock ~= the 6 MiB load stream's tail.
    with tc.tile_wait_until(0.025):
        for h in range(HPC - 1):
            ov = o_d[h].rearrange("(p j) d -> p j d", p=P)
            nc.sync.dma_start(ov[:], out_sbs[h][:])
        # last head: store per half so the final DMA is small and its
        # completion receipt starts as early as possible
        ov = o_d[HPC - 1].rearrange("(p j) d -> p j d", p=P)
        nc.sync.dma_start(ov[:, 0:8], out_sbs[HPC - 1][:, 0:8])
        nc.sync.dma_start(ov[:, 8:J], out_sbs[HPC - 1][:, 8:J])


def build():
    nc = bacc.Bacc("TRN2", target_bir_lowering=False, debug=False)
    qkv_d = nc.dram_tensor("qkv", [HPC, 3, L, D], F32, kind="ExternalInput").ap()
    o_d = nc.dram_tensor("out", [HPC, L, D], F32, kind="ExternalOutput").ap()
    with tile.TileContext(nc) as tc, ExitStack() as ctx:
        _body(ctx, tc, o_d, qkv_d)
    nc.compile()
    return nc


_NC = None


def _get_nc():
    global _NC
    if _NC is None:
        _NC = build()
    return _NC


def make_in_maps(q, k, v):
    qf = np.asarray(q, dtype=np.float32).reshape(B * H, L, D)
    kf = np.asarray(k, dtype=np.float32).reshape(B * H, L, D)
    vf = np.asarray(v, dtype=np.float32).reshape(B * H, L, D)
    # [B*H, 3, L, D]: per head q/k/v adjacent so one DMA loads a whole head.
    qkv = np.stack([qf, kf, vf], axis=1)
    return [
        {"qkv": np.ascontiguousarray(qkv[c * HPC : (c + 1) * HPC])}
        for c in range(N_CORES)
    ]


def run_sharded(q, k, v, **spmd_kwargs):
    """Run on all 8 cores; returns (full_output, BassKernelResults)."""
    nc = _get_nc()
    res = run_bass_kernel_spmd(
        nc, make_in_maps(q, k, v), core_ids=list(range(N_CORES)), **spmd_kwargs
    )
    shards = [np.asarray(res.results[c]["out"]) for c in range(N_CORES)]
    out = np.concatenate(shards, axis=0).reshape(B, H, L, D).astype(np.float32)
    return out, res


def kernel(q, k, v):
    out, _ = run_sharded(q, k, v)
    return out
